# revision 2
# baseline (speedup 1.0000x reference)
"""Trainium2 Bass kernel for:
    tgt_norm = tgt / ||tgt||_2 (rows)
    sim      = tgt_norm @ tgt_norm.T          (per batch, NxN)
    out      = tanh(sim) @ tgt                (per batch, NxD)

Sharding: data-parallel over batch B=8, one batch per NeuronCore.
Per-core fused flash-style kernel: sim is never materialized in HBM.

Self-contained: only needs the concourse tree staged on the machine.
"""

import sys

for _p in ("/opt/trn_rl_repo",):
    if _p not in sys.path:
        sys.path.append(_p)

import numpy as np

import concourse.bacc as bacc
import concourse.mybir as mybir
import concourse.tile as tile
from concourse.bass_utils import run_bass_kernel_spmd

P = 128  # partitions

F32 = mybir.dt.float32
BF16 = mybir.dt.bfloat16
AF = mybir.ActivationFunctionType


def build_kernel(N=4096, D=512, QB=512):
    """One NeuronCore program: tgt [N, D] f32 -> out [N, D] f32."""
    NT = N // P   # key tiles (128 rows each)
    DC = D // P   # contraction chunks of the feature dim
    NQ = N // QB  # query blocks
    XQ = QB // P  # 128-row sub-blocks per query block

    nc = bacc.Bacc(debug=False)
    tgt = nc.dram_tensor("tgt", [N, D], F32, kind="ExternalInput")
    out = nc.dram_tensor("out", [N, D], F32, kind="ExternalOutput")

    with tile.TileContext(nc) as tc:
        with (
            tc.tile_pool(name="persist", bufs=1) as pb,
            tc.tile_pool(name="sq", bufs=2) as sqp,
            tc.tile_pool(name="stage", bufs=3) as stp,
            tc.tile_pool(name="tanh", bufs=3) as thp,
            tc.tile_pool(name="osb", bufs=3) as osp,
            tc.tile_pool(name="ps_out", bufs=4, space="PSUM") as pso,
            tc.tile_pool(name="ps_sim", bufs=3, space="PSUM") as pss,
        ):
            # ---------------- phase 1: load, norms, casts, transpose -------
            big = pb.tile([P, NT * D], F32)       # tgt, N-major f32
            tgtb = pb.tile([P, NT * D], BF16)     # tgt, N-major bf16
            tnT = pb.tile([P, DC * N], BF16)      # normalized tgt, D-major bf16
            sums = pb.tile([P, NT], F32)
            norm = pb.tile([P, NT], F32)
            inv = pb.tile([P, NT], F32)

            tnT_v = tnT[:].rearrange("p (c n) -> p c n", c=DC)
            tgtb_v = tgtb[:].rearrange("p (t d) -> p t d", t=NT)

            for j in range(NT):
                sl = big[:, j * D:(j + 1) * D]
                nc.sync.dma_start(sl, tgt[j * P:(j + 1) * P, :])
                sq = sqp.tile([P, D], F32)
                nc.scalar.activation(sq[:], sl, AF.Square,
                                     accum_out=sums[:, j:j + 1])

            nc.scalar.sqrt(norm[:], sums[:])
            nc.vector.reciprocal(inv[:], norm[:])

            for j in range(NT):
                sl = big[:, j * D:(j + 1) * D]
                stg = stp.tile([P, D], BF16)
                nc.vector.tensor_scalar_mul(stg[:], sl, inv[:, j:j + 1])
                nc.sync.dma_start_transpose(tnT_v[:, :, j * P:(j + 1) * P], stg[:])
                nc.vector.tensor_copy(tgtb[:, j * D:(j + 1) * D], sl)

            # ---------------- phase 2: fused sim -> tanh -> out ------------
            for qi in range(NQ):
                out_ps = [pso.tile([P, D], F32, tag="ops", name=f"ops{x}")
                          for x in range(XQ)]

                def out_mms(kj, th):
                    for x in range(XQ):
                        nc.tensor.matmul(
                            out_ps[x][:],
                            th[:, x * P:(x + 1) * P],
                            tgtb_v[:, kj, :],
                            start=(kj == 0), stop=(kj == NT - 1),
                        )

                prev = None
                for kj in range(NT):
                    sim_ps = pss.tile([P, QB], F32)
                    for c in range(DC):
                        nc.tensor.matmul(
                            sim_ps[:],
                            tnT_v[:, c, kj * P:(kj + 1) * P],
                            tnT_v[:, c, qi * QB:(qi + 1) * QB],
                            start=(c == 0), stop=(c == DC - 1),
                        )
                    if prev is not None:
                        out_mms(kj - 1, prev)
                    th = thp.tile([P, QB], BF16)
                    nc.scalar.activation(th[:], sim_ps[:], AF.Tanh)
                    prev = th
                out_mms(NT - 1, prev)

                for x in range(XQ):
                    ob = osp.tile([P, D], F32)
                    nc.vector.tensor_copy(ob[:], out_ps[x][:])
                    r0 = qi * QB + x * P
                    nc.sync.dma_start(out[r0:r0 + P, :], ob[:])

    nc.compile()
    return nc


_cache = {}


def _get_nc(N, D):
    key = (N, D)
    if key not in _cache:
        _cache[key] = build_kernel(N, D)
    return _cache[key]


def _run(tgt, trace=False):
    """tgt: [B, N, D] f32. Returns (out [B, N, D] f32, exec_time_ns|None)."""
    tgt = np.ascontiguousarray(np.asarray(tgt, dtype=np.float32))
    B, N, D = tgt.shape
    nc = _get_nc(N, D)
    in_maps = [{"tgt": tgt[b]} for b in range(B)]
    res = run_bass_kernel_spmd(nc, in_maps, core_ids=list(range(B)), trace=trace)
    outp = np.stack([res.results[b]["out"] for b in range(B)], axis=0)
    return outp.astype(np.float32), res.exec_time_ns


def kernel(tgt, query_pos=None, objects_num=None, **_unused):
    out, _ = _run(tgt, trace=False)
    return out
